# revision 12
# baseline (speedup 1.0000x reference)
"""BitLinear158 forward on 8 Trainium2 NeuronCores.

y = x @ quantize(W).T where quantize is the absmean ternary quantizer:
    gamma = mean(|W|) + 1e-6 ; qw = sign(W) * min(round(|W/gamma|), 1)

Strategy (tensor parallel over out_features, x replicated):
  - host: quantize W to ternary exactly (bit-exact replication of the
    reference quantizer) and cast to fp8 e4m3 ({-1,0,1} are exact in fp8).
  - x is cast to fp8 e4m3 (hi). For the first N_CORR of the 32
    contraction slabs (128 k each) the e4m3 residual lo = e4m3(x - hi)
    is also computed, so those slabs contribute ~2^-8 relative error
    while the rest carry plain e4m3 error (~2.7e-2 rms). Measured on the
    problem data (fixed jax key 0, fully deterministic; HW runs
    reproduce the host-simulated error to 4 digits), the end-to-end
    scale-relative absmax error is
      N_CORR=0 -> 1.648e-2, 8 -> 1.470e-2, 16 -> 1.143e-2, 32 -> 5.1e-4
    against the 2e-2 harness gate. Measured HW exec time:
      N_CORR=0 -> 910211 ns, 8 -> 1133581 ns, 16 -> 1373956 ns
    (vs 3620953 ns for the dual-bf16 baseline and ~1.82 ms for a single
    bf16 pass; pure-DR is ~99% of the 157 TF/s fp8 PE roofline).
  - each core runs fp8 DoubleRow matmuls (perf_mode=DoubleRow): each MM
    contracts 2 slabs (2x128 k) at the same per-MM cost as one bf16 MM
    (measured 222 ns at N=512), i.e. 2x bf16 MAC throughput. Per token
    tile and 512-wide output chunk: 16 hi MMs + N_CORR/2 lo MMs versus
    32 bf16 MMs for a plain bf16 kernel.
  - all layout work (transpose/tiling/dtype) happens on host; the device
    executes DMA + DoubleRow matmul + PSUM-copy only.
"""

import numpy as np
import ml_dtypes

import concourse.bass as bass
import concourse.bacc as bacc
import concourse.mybir as mybir
import concourse.tile as tile
from concourse import bass_utils

# Problem shapes (hardcoded per contract).
B, S, D_IN, D_OUT = 4, 2048, 4096, 16384
N_CORES = 8
O_PER = D_OUT // N_CORES          # 2048 out-features per core
T_TOK = B * S                     # 8192 tokens
KS = D_IN // 128                  # 32 contraction slabs of 128
T_TILES = T_TOK // 128            # 64 token tiles of 128
NCHUNK = O_PER // 512             # 4 PSUM chunks of 512 out-features
EPS = 1e-6

# Number of contraction slabs (of 32) that get the e4m3 lo-residual
# correction. Must be even. Measured scale-relative absmax error on the
# problem data (gate 2e-2):
#   0 -> 1.648e-2, 8 -> 1.470e-2, 12 -> 1.34e-2, 16 -> 1.14e-2
# (HW runs reproduce the host-simulated error to 4 digits; the data and
# arithmetic are fully deterministic.)
N_CORR = 0

# Set by test harness to capture profiling info; leave False for grading.
TRACE = False
TMPDIR = None
LAST_RESULTS = None

FP8 = ml_dtypes.float8_e4m3


def _quantize_ref(weight: np.ndarray) -> np.ndarray:
    """Replication of reference.absmean_quantize in numpy (verified
    bit-identical to the eager-jax reference on the problem data)."""
    gamma = np.float32(np.abs(weight).mean(dtype=np.float64)) + np.float32(EPS)
    ws = (weight / gamma).astype(np.float32)
    return (np.sign(ws) * np.minimum(np.round(np.abs(ws)), np.float32(1.0))
            ).astype(np.float32)


def build_program(n_corr: int) -> bass.Bass:
    """Emit the per-core Bass/Tile program.

    DRAM I/O (per core):
      xt [T_TILES, 128, (KS+n_corr)*128] fp8 -- x pre-tiled per token tile:
           [partition, slab, tok] with hi for all KS slabs then lo for the
           first n_corr slabs
      wT [KS*128, O_PER] fp8 -- this core's ternary W.T shard
      y  [T_TOK, O_PER]  f32 -- this core's output slice
    """
    ksx = KS + n_corr
    nc = bacc.Bacc("TRN2", target_bir_lowering=False, debug=False)
    xt_d = nc.dram_tensor("xt", [T_TILES, 128, ksx * 128], mybir.dt.float8e4,
                          kind="ExternalInput")
    wT_d = nc.dram_tensor("wT", [KS * 128, O_PER], mybir.dt.float8e4,
                          kind="ExternalInput")
    y_d = nc.dram_tensor("y", [T_TOK, O_PER], mybir.dt.float32,
                         kind="ExternalOutput")

    xtr = xt_d.ap().rearrange("n p (k t) -> n p k t", k=ksx)
    wTr = wT_d.ap().rearrange("(k p) o -> p k o", p=128)
    DR = mybir.MatmulPerfMode.DoubleRow

    with tile.TileContext(nc) as tc:
        with (
            tc.tile_pool(name="qw", bufs=1) as qw_pool,
            tc.tile_pool(name="xt", bufs=3) as xt_pool,
            tc.tile_pool(name="outs", bufs=2) as out_pool,
            tc.tile_pool(name="wu", bufs=1) as wu_pool,
            tc.tile_pool(name="psum", bufs=2, space="PSUM") as psum_pool,
        ):
            def x_load(t):
                xt = xt_pool.tile([128, ksx, 128], mybir.dt.float8e4,
                                  name="xt", tag="xt")
                nc.gpsimd.dma_start(out=xt, in_=xtr[t])
                return xt

            # x tiles go on the gpsimd DMA queue, weights on the sync queue
            # so they stream concurrently; both are issued first.
            x0 = x_load(0)
            x1 = x_load(1)

            # Resident ternary weight shard, DMA'd pair-by-pair so matmul
            # group g can start as soon as slab pair g has landed.
            qw = qw_pool.tile([128, KS, O_PER], mybir.dt.float8e4)
            for k in range(KS // 2):
                nc.sync.dma_start(out=qw[:, 2 * k:2 * k + 2, :],
                                  in_=wTr[:, 2 * k:2 * k + 2, :])

            # Warmup matmuls on zeroed dummy tiles: they run during the
            # ~12us framework preamble + first-DMA window and keep the PE's
            # HAM activity monitor busy, so the real matmuls start on a warm
            # 2.4 GHz clock instead of the cold-gated 1.2 GHz one.
            wu_x = wu_pool.tile([128, 2, 128], mybir.dt.float8e4)
            wu_w = wu_pool.tile([128, 2, 512], mybir.dt.float8e4)
            nc.vector.memset(wu_x, 0)
            nc.vector.memset(wu_w, 0)
            wu_ps = psum_pool.tile([128, 512], mybir.dt.float32,
                                   name="ps0", tag="ps0")
            for _ in range(20):
                nc.tensor.matmul(wu_ps, wu_x, wu_w, start=True, stop=True,
                                 perf_mode=DR)

            # (lhsT slab-pair offset in xt, w slab-pair index) per MM group
            groups = [(2 * sp, 2 * sp) for sp in range(KS // 2)] + \
                     [(KS + 2 * lp, 2 * lp) for lp in range(n_corr // 2)]
            ng = len(groups)

            def tile_mms(xtile, pss, gi):
                xs, wsl = groups[gi]
                for c in range(NCHUNK):
                    nc.tensor.matmul(
                        pss[c],
                        xtile[:, xs:xs + 2, :],
                        qw[:, wsl:wsl + 2, c * 512:(c + 1) * 512],
                        start=(gi == 0),
                        stop=(gi == ng - 1),
                        perf_mode=DR,
                    )

            def emit_out(t, pss):
                # PSUM -> SBUF copies split across scalar + vector engines,
                # per-chunk output DMA on the (by then idle) sync queue so
                # the tail drains as each chunk closes instead of all at the
                # end of the tile.
                ot = out_pool.tile([128, O_PER], mybir.dt.float32)
                for c in range(NCHUNK):
                    cs = slice(c * 512, (c + 1) * 512)
                    if c < 2:
                        nc.scalar.copy(out=ot[:, cs], in_=pss[c])
                    else:
                        nc.vector.tensor_copy(out=ot[:, cs], in_=pss[c])
                    q = nc.sync if c % 2 == 0 else nc.gpsimd
                    q.dma_start(
                        out=y_d.ap()[t * 128:(t + 1) * 128, cs],
                        in_=ot[:, cs],
                    )

            def new_pss():
                return [psum_pool.tile([128, 512], mybir.dt.float32,
                                       name=f"ps{c}", tag=f"ps{c}")
                        for c in range(NCHUNK)]

            # Token tiles 0 and 1 run paired, slab-pair-major, across all 8
            # PSUM banks: each arriving weight slab pair unlocks 8 matmuls
            # (~1.7us of PE work vs ~1.4us of DMA per pair), so the warm PE
            # is never starved while the weight shard streams in.
            pss0 = new_pss()
            pss1 = new_pss()
            for gi in range(ng):
                tile_mms(x0, pss0, gi)
                tile_mms(x1, pss1, gi)
            xcur = x_load(2)
            emit_out(0, pss0)
            emit_out(1, pss1)

            for t in range(2, T_TILES):
                xtile = xcur
                if t + 1 < T_TILES:
                    xcur = x_load(t + 1)
                pss = new_pss()
                for gi in range(ng):
                    tile_mms(xtile, pss, gi)
                emit_out(t, pss)
    nc.compile()
    return nc


def kernel(x: np.ndarray, weight: np.ndarray) -> np.ndarray:
    global LAST_RESULTS
    assert x.shape == (B, S, D_IN) and weight.shape == (D_OUT, D_IN)

    # Host-side prep: exact ternary quantize -> fp8; x hi/lo tiled fp8.
    qw = _quantize_ref(weight)
    qwT = np.ascontiguousarray(qw.T).astype(FP8)     # [D_IN, D_OUT]

    x2 = x.reshape(T_TOK, D_IN)
    xhi = x2.astype(FP8)
    xlo = (x2 - xhi.astype(np.float32)).astype(FP8)

    # xt[tile, p, slab, tt]: slabs 0..KS-1 = hi, KS..KS+N_CORR-1 = lo of
    # slabs 0..N_CORR-1.
    def tiled(a, ks):  # a: [T_TOK, ks*128] -> [T_TILES, 128, ks, 128]
        return (a.reshape(T_TILES, 128, ks, 128)    # [tile, tt, k, p]
                 .transpose(0, 3, 2, 1))            # [tile, p, k, tt]

    xt = np.concatenate(
        [tiled(xhi, KS), tiled(xlo[:, :N_CORR * 128], N_CORR)], axis=2,
    ).reshape(T_TILES, 128, (KS + N_CORR) * 128)
    xt = np.ascontiguousarray(xt)

    nc = build_program(N_CORR)
    in_maps = [
        {"xt": xt,
         "wT": np.ascontiguousarray(qwT[:, c * O_PER:(c + 1) * O_PER])}
        for c in range(N_CORES)
    ]
    res = bass_utils.run_bass_kernel_spmd(
        nc, in_maps, list(range(N_CORES)), trace=TRACE, tmpdir=TMPDIR,
    )
    LAST_RESULTS = res
    y = np.concatenate([res.results[c]["y"] for c in range(N_CORES)], axis=1)
    return np.ascontiguousarray(y.reshape(B, S, D_OUT).astype(np.float32,
                                                              copy=False))


# revision 16
# speedup vs baseline: 1.2058x; 1.2058x over previous
"""BitLinear158 forward on 8 Trainium2 NeuronCores.

y = x @ quantize(W).T where quantize is the absmean ternary quantizer:
    gamma = mean(|W|) + 1e-6 ; qw = sign(W) * min(round(|W/gamma|), 1)

Strategy (tensor parallel over out_features, x replicated):
  - host: quantize W to ternary exactly (bit-exact replication of the
    reference quantizer) and cast to fp8 e4m3 ({-1,0,1} are exact in fp8).
  - x is cast to fp8 e4m3 (hi). For the first N_CORR of the 32
    contraction slabs (128 k each) the e4m3 residual lo = e4m3(x - hi)
    is also computed, so those slabs contribute ~2^-8 relative error
    while the rest carry plain e4m3 error (~2.7e-2 rms). Measured on the
    problem data (fixed jax key 0, fully deterministic; HW runs
    reproduce the host-simulated error to 4 digits), the end-to-end
    scale-relative absmax error is
      N_CORR=0 -> 1.648e-2, 8 -> 1.470e-2, 16 -> 1.143e-2, 32 -> 5.1e-4
    against the 2e-2 harness gate. Measured HW exec time:
      N_CORR=0 -> 908532 ns, 8 -> 1133581 ns, 16 -> 1373956 ns
    (vs 3620953 ns for the dual-bf16 baseline and ~1.82 ms for a single
    bf16 pass; pure-DR is ~99% of the 157 TF/s fp8 PE roofline, with
    ~12us unavoidable framework preamble + ~7us tail).
  - each core runs fp8 DoubleRow matmuls (perf_mode=DoubleRow): each MM
    contracts 2 slabs (2x128 k) at the same per-MM cost as one bf16 MM
    (measured 222 ns at N=512), i.e. 2x bf16 MAC throughput. Per token
    tile and 512-wide output chunk: 16 hi MMs + N_CORR/2 lo MMs versus
    32 bf16 MMs for a plain bf16 kernel.
  - all layout work (transpose/tiling/dtype) happens on host; the device
    executes DMA + DoubleRow matmul + PSUM-copy only.
"""

import numpy as np
import ml_dtypes

import concourse.bass as bass
import concourse.bacc as bacc
import concourse.mybir as mybir
import concourse.tile as tile
from concourse import bass_utils

# Problem shapes (hardcoded per contract).
B, S, D_IN, D_OUT = 4, 2048, 4096, 16384
N_CORES = 8
O_PER = D_OUT // N_CORES          # 2048 out-features per core
T_TOK = B * S                     # 8192 tokens
KS = D_IN // 128                  # 32 contraction slabs of 128
T_TILES = T_TOK // 128            # 64 token tiles of 128
NCHUNK = O_PER // 512             # 4 PSUM chunks of 512 out-features
EPS = 1e-6

# Number of contraction slabs (of 32) that get the e4m3 lo-residual
# correction. Must be even. Measured scale-relative absmax error on the
# problem data (gate 2e-2):
#   0 -> 1.648e-2, 8 -> 1.470e-2, 12 -> 1.34e-2, 16 -> 1.14e-2
# (HW runs reproduce the host-simulated error to 4 digits; the data and
# arithmetic are fully deterministic.)
N_CORR = 0

# Set by test harness to capture profiling info; leave False for grading.
TRACE = False
TMPDIR = None
LAST_RESULTS = None

FP8 = ml_dtypes.float8_e4m3


def _quantize_ref(weight: np.ndarray) -> np.ndarray:
    """Replication of reference.absmean_quantize in numpy (verified
    bit-identical to the eager-jax reference on the problem data)."""
    gamma = np.float32(np.abs(weight).mean(dtype=np.float64)) + np.float32(EPS)
    ws = (weight / gamma).astype(np.float32)
    return (np.sign(ws) * np.minimum(np.round(np.abs(ws)), np.float32(1.0))
            ).astype(np.float32)


def build_program(n_corr: int) -> bass.Bass:
    """Emit the per-core Bass/Tile program.

    DRAM I/O (per core):
      xt [T_TILES, 128, (KS+n_corr)*128] fp8 -- x pre-tiled per token tile:
           [partition, slab, tok] with hi for all KS slabs then lo for the
           first n_corr slabs
      wT [KS*128, O_PER] fp8 -- this core's ternary W.T shard
      y  [T_TOK, O_PER]  f32 -- this core's output slice
    """
    ksx = KS + n_corr
    nc = bacc.Bacc("TRN2", target_bir_lowering=False, debug=False)
    xt_d = nc.dram_tensor("xt", [T_TILES, 128, ksx * 128], mybir.dt.float8e4,
                          kind="ExternalInput")
    wT_d = nc.dram_tensor("wT", [KS * 128, O_PER], mybir.dt.float8e4,
                          kind="ExternalInput")
    y_d = nc.dram_tensor("y", [T_TOK, O_PER], mybir.dt.float32,
                         kind="ExternalOutput")

    xtr = xt_d.ap().rearrange("n p (k t) -> n p k t", k=ksx)
    wTr = wT_d.ap().rearrange("(k p) o -> p k o", p=128)
    DR = mybir.MatmulPerfMode.DoubleRow

    with tile.TileContext(nc) as tc:
        with (
            tc.tile_pool(name="qw", bufs=1) as qw_pool,
            tc.tile_pool(name="xt", bufs=3) as xt_pool,
            tc.tile_pool(name="outs", bufs=2) as out_pool,
            tc.tile_pool(name="wu", bufs=1) as wu_pool,
            tc.tile_pool(name="psum", bufs=2, space="PSUM") as psum_pool,
        ):
            def x_load(t):
                xt = xt_pool.tile([128, ksx, 128], mybir.dt.float8e4,
                                  name="xt", tag="xt")
                nc.gpsimd.dma_start(out=xt, in_=xtr[t])
                return xt

            # x tiles go on the gpsimd DMA queue, weights on the sync queue
            # so they stream concurrently; both are issued first. Tiles 0
            # and 1 are loaded as interleaved half-tiles so the first matmul
            # groups' stationary slabs land ~4us earlier than a whole-tile
            # load would allow.
            H = ksx // 2
            x0 = xt_pool.tile([128, ksx, 128], mybir.dt.float8e4,
                              name="xt", tag="xt")
            x1 = xt_pool.tile([128, ksx, 128], mybir.dt.float8e4,
                              name="xt", tag="xt")
            nc.gpsimd.dma_start(out=x0[:, :H, :], in_=xtr[0][:, :H, :])
            nc.gpsimd.dma_start(out=x1[:, :H, :], in_=xtr[1][:, :H, :])
            nc.gpsimd.dma_start(out=x0[:, H:, :], in_=xtr[0][:, H:, :])
            nc.gpsimd.dma_start(out=x1[:, H:, :], in_=xtr[1][:, H:, :])

            # Resident ternary weight shard, DMA'd pair-by-pair so matmul
            # group g can start as soon as slab pair g has landed.
            qw = qw_pool.tile([128, KS, O_PER], mybir.dt.float8e4)
            for k in range(KS // 2):
                nc.sync.dma_start(out=qw[:, 2 * k:2 * k + 2, :],
                                  in_=wTr[:, 2 * k:2 * k + 2, :])

            # Warmup matmuls on zeroed dummy tiles: they run during the
            # ~9us framework preamble + first-DMA window and keep the PE's
            # HAM activity monitor busy, so the real matmuls start on a warm
            # 2.4 GHz clock instead of the cold-gated 1.2 GHz one. Sized to
            # end right as the first half-tile + weight pair land.
            wu_x = wu_pool.tile([128, 2, 128], mybir.dt.float8e4)
            wu_w = wu_pool.tile([128, 2, 512], mybir.dt.float8e4)
            nc.vector.memset(wu_x, 0)
            nc.vector.memset(wu_w, 0)
            wu_ps = psum_pool.tile([128, 512], mybir.dt.float32,
                                   name="ps0", tag="ps0")
            for _ in range(10):
                nc.tensor.matmul(wu_ps, wu_x, wu_w, start=True, stop=True,
                                 perf_mode=DR)

            # (lhsT slab-pair offset in xt, w slab-pair index) per MM group
            groups = [(2 * sp, 2 * sp) for sp in range(KS // 2)] + \
                     [(KS + 2 * lp, 2 * lp) for lp in range(n_corr // 2)]
            ng = len(groups)

            def tile_mms(xtile, pss, gi):
                xs, wsl = groups[gi]
                for c in range(NCHUNK):
                    nc.tensor.matmul(
                        pss[c],
                        xtile[:, xs:xs + 2, :],
                        qw[:, wsl:wsl + 2, c * 512:(c + 1) * 512],
                        start=(gi == 0),
                        stop=(gi == ng - 1),
                        perf_mode=DR,
                    )

            def emit_out(t, pss):
                # PSUM -> SBUF copies split across scalar + vector engines,
                # per-chunk output DMA on the (by then idle) sync queue so
                # the tail drains as each chunk closes instead of all at the
                # end of the tile.
                ot = out_pool.tile([128, O_PER], mybir.dt.float32)
                for c in range(NCHUNK):
                    cs = slice(c * 512, (c + 1) * 512)
                    if c < 2:
                        nc.scalar.copy(out=ot[:, cs], in_=pss[c])
                    else:
                        nc.vector.tensor_copy(out=ot[:, cs], in_=pss[c])
                    nc.sync.dma_start(
                        out=y_d.ap()[t * 128:(t + 1) * 128, cs],
                        in_=ot[:, cs],
                    )

            def new_pss():
                return [psum_pool.tile([128, 512], mybir.dt.float32,
                                       name=f"ps{c}", tag=f"ps{c}")
                        for c in range(NCHUNK)]

            # Token tiles 0 and 1 run paired, slab-pair-major, across all 8
            # PSUM banks: each arriving weight slab pair unlocks 8 matmuls
            # (~1.7us of PE work vs ~1.4us of DMA per pair), so the warm PE
            # is never starved while the weight shard streams in.
            pss0 = new_pss()
            pss1 = new_pss()
            for gi in range(ng):
                tile_mms(x0, pss0, gi)
                tile_mms(x1, pss1, gi)
            xcur = x_load(2)
            emit_out(0, pss0)
            emit_out(1, pss1)

            for t in range(2, T_TILES):
                xtile = xcur
                if t + 1 < T_TILES:
                    xcur = x_load(t + 1)
                pss = new_pss()
                for gi in range(ng):
                    tile_mms(xtile, pss, gi)
                emit_out(t, pss)
    nc.compile()
    return nc


def kernel(x: np.ndarray, weight: np.ndarray) -> np.ndarray:
    global LAST_RESULTS
    assert x.shape == (B, S, D_IN) and weight.shape == (D_OUT, D_IN)

    # Host-side prep: exact ternary quantize -> fp8; x hi/lo tiled fp8.
    qw = _quantize_ref(weight)
    qwT = np.ascontiguousarray(qw.T).astype(FP8)     # [D_IN, D_OUT]

    x2 = x.reshape(T_TOK, D_IN)
    xhi = x2.astype(FP8)
    xlo = (x2 - xhi.astype(np.float32)).astype(FP8)

    # xt[tile, p, slab, tt]: slabs 0..KS-1 = hi, KS..KS+N_CORR-1 = lo of
    # slabs 0..N_CORR-1.
    def tiled(a, ks):  # a: [T_TOK, ks*128] -> [T_TILES, 128, ks, 128]
        return (a.reshape(T_TILES, 128, ks, 128)    # [tile, tt, k, p]
                 .transpose(0, 3, 2, 1))            # [tile, p, k, tt]

    xt = np.concatenate(
        [tiled(xhi, KS), tiled(xlo[:, :N_CORR * 128], N_CORR)], axis=2,
    ).reshape(T_TILES, 128, (KS + N_CORR) * 128)
    xt = np.ascontiguousarray(xt)

    nc = build_program(N_CORR)
    in_maps = [
        {"xt": xt,
         "wT": np.ascontiguousarray(qwT[:, c * O_PER:(c + 1) * O_PER])}
        for c in range(N_CORES)
    ]
    res = bass_utils.run_bass_kernel_spmd(
        nc, in_maps, list(range(N_CORES)), trace=TRACE, tmpdir=TMPDIR,
    )
    LAST_RESULTS = res
    y = np.concatenate([res.results[c]["y"] for c in range(N_CORES)], axis=1)
    return np.ascontiguousarray(y.reshape(B, S, D_OUT).astype(np.float32,
                                                              copy=False))
